# revision 10
# baseline (speedup 1.0000x reference)
"""
Trainium2 Bass kernel for nn_MetaAttention.

Computation (per batch b):
    rowsum[h,i]     = sum_j m[b,h,i,j]
    aggregated[i,j] = sum_h rowsum[h,i] * m[b,h,i,j]
    out[b]          = softmax(aggregated.flatten()).reshape(N, N)

Sharding: pure data parallel over B=16 across 8 cores (2 batches/core).

Per-core strategy (memory regime, ~64 MB HBM traffic/core; the SDMA
engines sustain ~16.5 GB/s each on this descriptor pattern regardless
of queue/ring choice — measured identical on 1-core and 8-core runs —
so the kernel is structured to keep the single HWDGE load queue busy
end-to-end and to minimize the serial tail after the last load):
  - Row tiles of P=112 partitions; partition p holds CONTIGUOUS rows
    7p..7p+6 ("(p t) j") so DMA descriptors are 9-12 KB contiguous
    DRAM segments. Row permutation is transparent (math is
    row-independent; the store inverts the map).
  - Loads are per (batch, PSUM-chunk of 4/3 row tiles, head) on the SP
    HWDGE ring; deep mh buffering (10 bufs) keeps DMA ahead through
    the inter-batch softmax chain.
  - The scale-accumulate over heads is split across engine paths (fp32
    PE matmul is 2-pass + half-rate, so PE alone can't carry it): per
    (head, tile) units go to PE diag-matmul (PSUM), DVE fused
    scalar_tensor_tensor (SBUF), or ACT-mult + GPSIMD-add (SBUF), with
    rotating assignments so each h-step has a near-constant mix. A DVE
    add merges PSUM partials into the SBUF agg.
  - rowsums: half the heads via one multi-tile DVE tensor_reduce per
    (chunk, head), half via per-tile ACT activation+accum_out.
  - Softmax is ONLINE (flash-style): each tile is exponentiated
    against its per-row max right after its merge, overlapped with the
    next chunk's load stream; the global max M and sum S are folded
    into one per-row scalar c = exp(m_row - M)/S applied by the final
    per-tile scale. Tail after the last load = last head's units +
    3 merges/exps + a short reduce chain + 7 scales/stores.
  - Stores: batch 0 via SWDGE (gpsimd) mid-stream; final batch via the
    then-idle SP HWDGE ring.
"""

import numpy as np

B, H, N = 16, 12, 784
NCORES = 8
BPC = B // NCORES          # batches per core
P = 112                    # partition tile (784 = 7 * 112)
NT = N // P                # 7 row tiles
CHUNKS = [(0, 4), (4, 2), (6, 1)]  # (first row-tile, n row-tiles) per PSUM chunk
JSPLITS = [(0, 512), (512, 272)]  # matmul free-dim splits (PSUM bank aligned)
ROWSUM_DVE_H = {0, 2, 4, 6, 8, 10}  # multi-tile DVE reduce; rest per-tile ACT

LAST_RESULT = None  # BassKernelResults of the most recent kernel() call


def unit_path(h, k, tail_chunk=False):
    """Engine path of the scale-accumulate unit (head h, row tile k):
    'pe' | 'init' | 'dve' | 'gps'. Normally 6 PE heads per tile,
    rotating by k; SBUF chain runs init -> dve -> dve -> gps -> gps ->
    gps (one cross-engine hop per tile). For the final chunk of the
    last batch (tail_chunk) the slow GPSIMD links are excluded so the
    post-last-load tail is short: 8 PE heads, init + 3 DVE.
    """
    npe = 8 if tail_chunk else 6
    if (h + 2 * k) % 12 < npe:
        return "pe"
    sbuf_heads = [hh for hh in range(H) if (hh + 2 * k) % 12 >= npe]
    idx = sbuf_heads.index(h)
    if idx == 0:
        return "init"
    if tail_chunk:
        return "dve"
    return "dve" if idx <= 2 else "gps"


def build_program():
    import concourse.bacc as bacc
    import concourse.tile as tile
    from concourse import mybir
    from concourse import bass_isa

    f32 = mybir.dt.float32
    nc = bacc.Bacc("TRN2")

    x = nc.dram_tensor("x", [BPC, H, N, N], f32, kind="ExternalInput")
    ident = nc.dram_tensor("ident", [P, P], f32, kind="ExternalInput")
    y = nc.dram_tensor("y", [BPC, N, N], f32, kind="ExternalOutput")

    with tile.TileContext(nc) as tc:
        with (
            tc.tile_pool(name="mh", bufs=10) as mh_pool,
            tc.tile_pool(name="agg", bufs=2) as agg_pool,
            tc.tile_pool(name="acc", bufs=4, space="PSUM") as acc_pool,
            tc.tile_pool(name="diag", bufs=4) as diag_pool,
            tc.tile_pool(name="scratch", bufs=4) as scratch_pool,
            tc.tile_pool(name="small", bufs=8) as small_pool,
            tc.tile_pool(name="consts", bufs=1) as const_pool,
        ):
            ident_sb = const_pool.tile([P, P], f32)
            nc.sync.dma_start(out=ident_sb, in_=ident[:, :])
            ones_sb = const_pool.tile([P, P], f32)
            nc.vector.memset(ones_sb, 1.0)

            for b in range(BPC):
                agg = agg_pool.tile([P, NT, N], f32, tag="agg")
                maxs = small_pool.tile([P, NT], f32, tag="maxs")
                negmaxs = small_pool.tile([P, NT], f32, tag="negmaxs")
                sums = small_pool.tile([P, NT], f32, tag="sums")

                for ci, (c0, ct) in enumerate(CHUNKS):
                    tailc = (b == BPC - 1) and (ci == len(CHUNKS) - 1)
                    accs = [
                        acc_pool.tile([P, 1024], f32, tag="acc",
                                      name=f"acc_{b}_{c0}_{k}")
                        for k in range(ct)
                    ]
                    pe_first = {
                        k: min(h for h in range(H)
                               if unit_path(h, c0 + k, tailc) == "pe")
                        for k in range(ct)
                    }
                    pe_last = {
                        k: max(h for h in range(H)
                               if unit_path(h, c0 + k, tailc) == "pe")
                        for k in range(ct)
                    }
                    for h in range(H):
                        mh = mh_pool.tile([P, ct, N], f32, tag="mh")
                        # partition p <- contiguous rows 7p..7p+6 of m[b,h]
                        src = x[b, h].rearrange("(p t) j -> p t j", p=P)
                        # max_dma_last_dim=N keeps every DMA descriptor a
                        # single-packet 3136-B row: multi-packet descriptors
                        # stream at ~17 GB/s/engine vs ~25 GB/s single-packet
                        nc.sync.dma_start(out=mh, in_=src[:, c0 : c0 + ct, :],
                                          max_dma_last_dim=N)

                        if h in ROWSUM_DVE_H:
                            rs7 = small_pool.tile([P, ct], f32, tag="rs7")
                            nc.vector.tensor_reduce(
                                out=rs7, in_=mh, axis=mybir.AxisListType.X,
                                op=mybir.AluOpType.add,
                            )
                            rs_of = lambda k: rs7[:, k : k + 1]
                        else:
                            rs7a = small_pool.tile([P, ct], f32, tag="rs7a")
                            scr = scratch_pool.tile([P, N], f32, tag="scr")
                            for k in range(ct):
                                nc.scalar.activation(
                                    out=scr, in_=mh[:, k, :],
                                    func=mybir.ActivationFunctionType.Copy,
                                    bias=0.0, scale=1.0,
                                    accum_out=rs7a[:, k : k + 1],
                                )
                            rs_of = lambda k: rs7a[:, k : k + 1]

                        for k in range(ct):
                            it = c0 + k
                            rs = rs_of(k)
                            p_ = unit_path(h, it, tailc)
                            if p_ == "pe":
                                dg = diag_pool.tile([P, P], f32, tag="dg")
                                nc.vector.tensor_scalar_mul(
                                    out=dg, in0=ident_sb, scalar1=rs
                                )
                                for j0, jn in JSPLITS:
                                    nc.tensor.matmul(
                                        accs[k][:, j0 : j0 + jn],
                                        lhsT=dg,
                                        rhs=mh[:, k, j0 : j0 + jn],
                                        start=(h == pe_first[k]),
                                        stop=(h == pe_last[k]),
                                    )
                            elif p_ == "init":
                                nc.vector.tensor_scalar_mul(
                                    out=agg[:, it, :], in0=mh[:, k, :], scalar1=rs
                                )
                            elif p_ == "dve":
                                nc.vector.scalar_tensor_tensor(
                                    out=agg[:, it, :],
                                    in0=mh[:, k, :],
                                    scalar=rs,
                                    in1=agg[:, it, :],
                                    op0=mybir.AluOpType.mult,
                                    op1=mybir.AluOpType.add,
                                )
                            else:  # gps: scale on ACT, add on gpsimd
                                sc2 = scratch_pool.tile([P, N], f32, tag="sc2")
                                nc.scalar.activation(
                                    out=sc2, in_=mh[:, k, :],
                                    func=mybir.ActivationFunctionType.Copy,
                                    bias=0.0, scale=rs,
                                )
                                nc.gpsimd.tensor_tensor(
                                    out=agg[:, it, :],
                                    in0=sc2,
                                    in1=agg[:, it, :],
                                    op=mybir.AluOpType.add,
                                )
                    # merge PSUM partial into agg; per-tile max; ONLINE exp
                    # against the per-row max, overlapped with the next
                    # chunk's load stream. sums[:, it] = per-row expsum.
                    for k in range(ct):
                        it = c0 + k
                        nc.vector.tensor_add(
                            out=agg[:, it, :],
                            in0=agg[:, it, :],
                            in1=accs[k][:, 0:N],
                        )
                        nc.vector.tensor_reduce(
                            out=maxs[:, it : it + 1],
                            in_=agg[:, it, :],
                            axis=mybir.AxisListType.X,
                            op=mybir.AluOpType.max,
                        )
                        nc.vector.tensor_scalar_mul(
                            out=negmaxs[:, it : it + 1],
                            in0=maxs[:, it : it + 1],
                            scalar1=-1.0,
                        )
                        nc.scalar.activation(
                            out=agg[:, it, :],
                            in_=agg[:, it, :],
                            func=mybir.ActivationFunctionType.Exp,
                            bias=negmaxs[:, it : it + 1],
                            scale=1.0,
                            accum_out=sums[:, it : it + 1],
                        )

                # ---- global softmax fix-up over the [N, N] of this batch:
                # c[row] = exp(m_row - M) / S folded into the final scale.
                # -M is computed directly as a MIN-reduce of negmaxs (saves
                # a negation hop on the serial chain) ----
                m1n = small_pool.tile([P, 1], f32, tag="m1n")
                nc.vector.tensor_reduce(
                    out=m1n, in_=negmaxs, axis=mybir.AxisListType.X,
                    op=mybir.AluOpType.min,
                )
                # cross-partition: PE transpose -> free-axis reduce ->
                # K=1 all-ones matmul broadcast (low latency; gpsimd
                # partition_all_reduce costs ~5us of Q7 dispatch)
                tps = acc_pool.tile([1, P], f32, tag="acc", name=f"tps_{b}")
                nc.tensor.transpose(tps, m1n, ident_sb)
                gmn = small_pool.tile([1, 1], f32, tag="gmn")
                nc.vector.tensor_reduce(
                    out=gmn, in_=tps, axis=mybir.AxisListType.X,
                    op=mybir.AluOpType.min,
                )
                bpsn = acc_pool.tile([P, 1], f32, tag="acc", name=f"bps_{b}")
                nc.tensor.matmul(bpsn, lhsT=ones_sb[0:1, :], rhs=gmn,
                                 start=True, stop=True)

                negM = small_pool.tile([P, 1], f32, tag="negM")
                nc.vector.tensor_scalar_mul(out=negM, in0=bpsn, scalar1=1.0)

                # E = exp(maxs - M)  [P, NT]
                escale = small_pool.tile([P, NT], f32, tag="escale")
                nc.scalar.activation(
                    out=escale, in_=maxs,
                    func=mybir.ActivationFunctionType.Exp,
                    bias=negM, scale=1.0,
                )
                # w = E * sums; S = cross-partition+tile sum of w
                wsum = small_pool.tile([P, NT], f32, tag="wsum")
                nc.vector.tensor_tensor(
                    out=wsum, in0=escale, in1=sums, op=mybir.AluOpType.mult,
                )
                s1 = small_pool.tile([P, 1], f32, tag="s1")
                nc.vector.tensor_reduce(
                    out=s1, in_=wsum, axis=mybir.AxisListType.X,
                    op=mybir.AluOpType.add,
                )
                # cross-partition sum + broadcast in one all-ones matmul
                sps = acc_pool.tile([P, 1], f32, tag="acc", name=f"sps_{b}")
                nc.tensor.matmul(sps, lhsT=ones_sb, rhs=s1, start=True, stop=True)
                rinv = small_pool.tile([P, 1], f32, tag="rinv")
                nc.vector.reciprocal(out=rinv, in_=sps)
                # c = E * (1/S)  [P, NT]
                csc = small_pool.tile([P, NT], f32, tag="csc")
                nc.vector.tensor_scalar_mul(out=csc, in0=escale, scalar1=rinv)

                # per-tile scale + store so the tail pipelines; alternate
                # the scale between ACT and DVE to halve its serial latency
                dst = y[b].rearrange("(p t) j -> p t j", p=P)
                store_eng = nc.sync if b == BPC - 1 else nc.gpsimd
                for it in range(NT):
                    if it % 2 == 0:
                        nc.scalar.activation(
                            out=agg[:, it, :],
                            in_=agg[:, it, :],
                            func=mybir.ActivationFunctionType.Copy,
                            bias=0.0,
                            scale=csc[:, it : it + 1],
                        )
                    else:
                        nc.vector.tensor_scalar_mul(
                            out=agg[:, it, :], in0=agg[:, it, :],
                            scalar1=csc[:, it : it + 1],
                        )
                    store_eng.dma_start(
                        out=dst[:, it, :], in_=agg[:, it, :]
                    )

    nc.finalize()  # Bacc: register alloc, nop/event-sem legalization, ISA codegen
    return nc


def kernel(mha_masks) -> np.ndarray:
    global LAST_RESULT
    from concourse.bass_utils import run_bass_kernel_spmd

    xfull = np.ascontiguousarray(np.asarray(mha_masks, dtype=np.float32))
    assert xfull.shape == (B, H, N, N), xfull.shape

    nc = build_program()
    ident = np.eye(P, dtype=np.float32)
    in_maps = [
        {"x": xfull[i * BPC : (i + 1) * BPC], "ident": ident}
        for i in range(NCORES)
    ]
    import os

    kw = {}
    if os.environ.get("KERNEL_TRACE_DIR"):
        kw = dict(trace=True, tmpdir=os.environ["KERNEL_TRACE_DIR"])
    res = run_bass_kernel_spmd(nc, in_maps, core_ids=list(range(NCORES)), **kw)
    LAST_RESULT = res
    out = np.concatenate(
        [np.asarray(r["y"], dtype=np.float32) for r in res.results], axis=0
    )
    return out


# revision 11
# speedup vs baseline: 1.0210x; 1.0210x over previous
"""
Trainium2 Bass kernel for nn_MetaAttention.

Computation (per batch b):
    rowsum[h,i]     = sum_j m[b,h,i,j]
    aggregated[i,j] = sum_h rowsum[h,i] * m[b,h,i,j]
    out[b]          = softmax(aggregated.flatten()).reshape(N, N)

Sharding: pure data parallel over B=16 across 8 cores (2 batches/core).

Per-core strategy (memory regime, ~64 MB HBM traffic/core; the SDMA
engines sustain ~16.5 GB/s each on this descriptor pattern regardless
of queue/ring choice — measured identical on 1-core and 8-core runs —
so the kernel is structured to keep the single HWDGE load queue busy
end-to-end and to minimize the serial tail after the last load):
  - Row tiles of P=112 partitions; partition p holds CONTIGUOUS rows
    7p..7p+6 ("(p t) j") so DMA descriptors are 9-12 KB contiguous
    DRAM segments. Row permutation is transparent (math is
    row-independent; the store inverts the map).
  - Loads are per (batch, PSUM-chunk of 4/3 row tiles, head) on the SP
    HWDGE ring; deep mh buffering (10 bufs) keeps DMA ahead through
    the inter-batch softmax chain.
  - The scale-accumulate over heads is split across engine paths (fp32
    PE matmul is 2-pass + half-rate, so PE alone can't carry it): per
    (head, tile) units go to PE diag-matmul (PSUM), DVE fused
    scalar_tensor_tensor (SBUF), or ACT-mult + GPSIMD-add (SBUF), with
    rotating assignments so each h-step has a near-constant mix. A DVE
    add merges PSUM partials into the SBUF agg.
  - rowsums: half the heads via one multi-tile DVE tensor_reduce per
    (chunk, head), half via per-tile ACT activation+accum_out.
  - Softmax is ONLINE (flash-style): each tile is exponentiated
    against its per-row max right after its merge, overlapped with the
    next chunk's load stream; the global max M and sum S are folded
    into one per-row scalar c = exp(m_row - M)/S applied by the final
    per-tile scale. Tail after the last load = last head's units +
    3 merges/exps + a short reduce chain + 7 scales/stores.
  - Stores: batch 0 via SWDGE (gpsimd) mid-stream; final batch via the
    then-idle SP HWDGE ring.
"""

import numpy as np

B, H, N = 16, 12, 784
NCORES = 8
BPC = B // NCORES          # batches per core
P = 112                    # partition tile (784 = 7 * 112)
NT = N // P                # 7 row tiles
CHUNKS = [(0, 4), (4, 2), (6, 1)]  # (first row-tile, n row-tiles) per PSUM chunk
JSPLITS = [(0, 512), (512, 272)]  # matmul free-dim splits (PSUM bank aligned)
ROWSUM_DVE_H = {0, 2, 4, 6, 8, 10}  # multi-tile DVE reduce; rest per-tile ACT

LAST_RESULT = None  # BassKernelResults of the most recent kernel() call


def unit_path(h, k, tail_chunk=False):
    """Engine path of the scale-accumulate unit (head h, row tile k):
    'pe' | 'init' | 'dve' | 'gps'. Normally 6 PE heads per tile,
    rotating by k; SBUF chain runs init -> dve -> dve -> gps -> gps ->
    gps (one cross-engine hop per tile). For the final chunk of the
    last batch (tail_chunk) the slow GPSIMD links are excluded so the
    post-last-load tail is short: 8 PE heads, init + 3 DVE.
    """
    npe = 8 if tail_chunk else 6
    if (h + 2 * k) % 12 < npe:
        return "pe"
    sbuf_heads = [hh for hh in range(H) if (hh + 2 * k) % 12 >= npe]
    idx = sbuf_heads.index(h)
    if idx == 0:
        return "init"
    if tail_chunk:
        return "dve"
    return "dve" if idx <= 2 else "gps"


def build_program():
    import concourse.bacc as bacc
    import concourse.tile as tile
    from concourse import mybir
    from concourse import bass_isa

    f32 = mybir.dt.float32
    nc = bacc.Bacc("TRN2")

    x = nc.dram_tensor("x", [BPC, H, N, N], f32, kind="ExternalInput")
    ident = nc.dram_tensor("ident", [P, P], f32, kind="ExternalInput")
    y = nc.dram_tensor("y", [BPC, N, N], f32, kind="ExternalOutput")

    with tile.TileContext(nc) as tc:
        with (
            tc.tile_pool(name="mh", bufs=10) as mh_pool,
            tc.tile_pool(name="agg", bufs=2) as agg_pool,
            tc.tile_pool(name="acc", bufs=4, space="PSUM") as acc_pool,
            tc.tile_pool(name="diag", bufs=4) as diag_pool,
            tc.tile_pool(name="scratch", bufs=4) as scratch_pool,
            tc.tile_pool(name="small", bufs=8) as small_pool,
            tc.tile_pool(name="consts", bufs=1) as const_pool,
        ):
            ident_sb = const_pool.tile([P, P], f32)
            nc.sync.dma_start(out=ident_sb, in_=ident[:, :])
            ones_sb = const_pool.tile([P, P], f32)
            nc.vector.memset(ones_sb, 1.0)

            for b in range(BPC):
                agg = agg_pool.tile([P, NT, N], f32, tag="agg")
                maxs = small_pool.tile([P, NT], f32, tag="maxs")
                negmaxs = small_pool.tile([P, NT], f32, tag="negmaxs")
                sums = small_pool.tile([P, NT], f32, tag="sums")

                for ci, (c0, ct) in enumerate(CHUNKS):
                    tailc = (b == BPC - 1) and (ci == len(CHUNKS) - 1)
                    accs = [
                        acc_pool.tile([P, 1024], f32, tag="acc",
                                      name=f"acc_{b}_{c0}_{k}")
                        for k in range(ct)
                    ]
                    pe_first = {
                        k: min(h for h in range(H)
                               if unit_path(h, c0 + k, tailc) == "pe")
                        for k in range(ct)
                    }
                    pe_last = {
                        k: max(h for h in range(H)
                               if unit_path(h, c0 + k, tailc) == "pe")
                        for k in range(ct)
                    }
                    for h in range(H):
                        mh = mh_pool.tile([P, ct, N], f32, tag="mh")
                        # partition p <- contiguous rows 7p..7p+6 of m[b,h]
                        src = x[b, h].rearrange("(p t) j -> p t j", p=P)
                        nc.sync.dma_start(out=mh, in_=src[:, c0 : c0 + ct, :])

                        if h in ROWSUM_DVE_H:
                            rs7 = small_pool.tile([P, ct], f32, tag="rs7")
                            nc.vector.tensor_reduce(
                                out=rs7, in_=mh, axis=mybir.AxisListType.X,
                                op=mybir.AluOpType.add,
                            )
                            rs_of = lambda k: rs7[:, k : k + 1]
                        else:
                            rs7a = small_pool.tile([P, ct], f32, tag="rs7a")
                            scr = scratch_pool.tile([P, N], f32, tag="scr")
                            for k in range(ct):
                                nc.scalar.activation(
                                    out=scr, in_=mh[:, k, :],
                                    func=mybir.ActivationFunctionType.Copy,
                                    bias=0.0, scale=1.0,
                                    accum_out=rs7a[:, k : k + 1],
                                )
                            rs_of = lambda k: rs7a[:, k : k + 1]

                        for k in range(ct):
                            it = c0 + k
                            rs = rs_of(k)
                            p_ = unit_path(h, it, tailc)
                            if p_ == "pe":
                                dg = diag_pool.tile([P, P], f32, tag="dg")
                                nc.vector.tensor_scalar_mul(
                                    out=dg, in0=ident_sb, scalar1=rs
                                )
                                for j0, jn in JSPLITS:
                                    nc.tensor.matmul(
                                        accs[k][:, j0 : j0 + jn],
                                        lhsT=dg,
                                        rhs=mh[:, k, j0 : j0 + jn],
                                        start=(h == pe_first[k]),
                                        stop=(h == pe_last[k]),
                                    )
                            elif p_ == "init":
                                nc.vector.tensor_scalar_mul(
                                    out=agg[:, it, :], in0=mh[:, k, :], scalar1=rs
                                )
                            elif p_ == "dve":
                                nc.vector.scalar_tensor_tensor(
                                    out=agg[:, it, :],
                                    in0=mh[:, k, :],
                                    scalar=rs,
                                    in1=agg[:, it, :],
                                    op0=mybir.AluOpType.mult,
                                    op1=mybir.AluOpType.add,
                                )
                            else:  # gps: scale on ACT, add on gpsimd
                                sc2 = scratch_pool.tile([P, N], f32, tag="sc2")
                                nc.scalar.activation(
                                    out=sc2, in_=mh[:, k, :],
                                    func=mybir.ActivationFunctionType.Copy,
                                    bias=0.0, scale=rs,
                                )
                                nc.gpsimd.tensor_tensor(
                                    out=agg[:, it, :],
                                    in0=sc2,
                                    in1=agg[:, it, :],
                                    op=mybir.AluOpType.add,
                                )
                    # merge PSUM partial into agg; per-tile max; ONLINE exp
                    # against the per-row max, overlapped with the next
                    # chunk's load stream. sums[:, it] = per-row expsum.
                    for k in range(ct):
                        it = c0 + k
                        nc.vector.tensor_add(
                            out=agg[:, it, :],
                            in0=agg[:, it, :],
                            in1=accs[k][:, 0:N],
                        )
                        nc.vector.tensor_reduce(
                            out=maxs[:, it : it + 1],
                            in_=agg[:, it, :],
                            axis=mybir.AxisListType.X,
                            op=mybir.AluOpType.max,
                        )
                        nc.vector.tensor_scalar_mul(
                            out=negmaxs[:, it : it + 1],
                            in0=maxs[:, it : it + 1],
                            scalar1=-1.0,
                        )
                        nc.scalar.activation(
                            out=agg[:, it, :],
                            in_=agg[:, it, :],
                            func=mybir.ActivationFunctionType.Exp,
                            bias=negmaxs[:, it : it + 1],
                            scale=1.0,
                            accum_out=sums[:, it : it + 1],
                        )

                # ---- global softmax fix-up over the [N, N] of this batch:
                # c[row] = exp(m_row - M) / S folded into the final scale.
                # -M is computed directly as a MIN-reduce of negmaxs (saves
                # a negation hop on the serial chain) ----
                m1n = small_pool.tile([P, 1], f32, tag="m1n")
                nc.vector.tensor_reduce(
                    out=m1n, in_=negmaxs, axis=mybir.AxisListType.X,
                    op=mybir.AluOpType.min,
                )
                # cross-partition: PE transpose -> free-axis reduce ->
                # K=1 all-ones matmul broadcast (low latency; gpsimd
                # partition_all_reduce costs ~5us of Q7 dispatch)
                tps = acc_pool.tile([1, P], f32, tag="acc", name=f"tps_{b}")
                nc.tensor.transpose(tps, m1n, ident_sb)
                gmn = small_pool.tile([1, 1], f32, tag="gmn")
                nc.vector.tensor_reduce(
                    out=gmn, in_=tps, axis=mybir.AxisListType.X,
                    op=mybir.AluOpType.min,
                )
                bpsn = acc_pool.tile([P, 1], f32, tag="acc", name=f"bps_{b}")
                nc.tensor.matmul(bpsn, lhsT=ones_sb[0:1, :], rhs=gmn,
                                 start=True, stop=True)

                negM = small_pool.tile([P, 1], f32, tag="negM")
                nc.vector.tensor_scalar_mul(out=negM, in0=bpsn, scalar1=1.0)

                # E = exp(maxs - M)  [P, NT]
                escale = small_pool.tile([P, NT], f32, tag="escale")
                nc.scalar.activation(
                    out=escale, in_=maxs,
                    func=mybir.ActivationFunctionType.Exp,
                    bias=negM, scale=1.0,
                )
                # w = E * sums; S = cross-partition+tile sum of w
                wsum = small_pool.tile([P, NT], f32, tag="wsum")
                nc.vector.tensor_tensor(
                    out=wsum, in0=escale, in1=sums, op=mybir.AluOpType.mult,
                )
                s1 = small_pool.tile([P, 1], f32, tag="s1")
                nc.vector.tensor_reduce(
                    out=s1, in_=wsum, axis=mybir.AxisListType.X,
                    op=mybir.AluOpType.add,
                )
                # cross-partition sum + broadcast in one all-ones matmul
                sps = acc_pool.tile([P, 1], f32, tag="acc", name=f"sps_{b}")
                nc.tensor.matmul(sps, lhsT=ones_sb, rhs=s1, start=True, stop=True)
                rinv = small_pool.tile([P, 1], f32, tag="rinv")
                nc.vector.reciprocal(out=rinv, in_=sps)
                # c = E * (1/S)  [P, NT]
                csc = small_pool.tile([P, NT], f32, tag="csc")
                nc.vector.tensor_scalar_mul(out=csc, in0=escale, scalar1=rinv)

                # per-tile scale + store so the tail pipelines; alternate
                # the scale between ACT and DVE to halve its serial latency
                dst = y[b].rearrange("(p t) j -> p t j", p=P)
                store_eng = nc.sync if b == BPC - 1 else nc.gpsimd
                for it in range(NT):
                    if it % 2 == 0:
                        nc.scalar.activation(
                            out=agg[:, it, :],
                            in_=agg[:, it, :],
                            func=mybir.ActivationFunctionType.Copy,
                            bias=0.0,
                            scale=csc[:, it : it + 1],
                        )
                    else:
                        nc.vector.tensor_scalar_mul(
                            out=agg[:, it, :], in0=agg[:, it, :],
                            scalar1=csc[:, it : it + 1],
                        )
                    store_eng.dma_start(
                        out=dst[:, it, :], in_=agg[:, it, :]
                    )

    nc.finalize()  # Bacc: register alloc, nop/event-sem legalization, ISA codegen
    return nc


def kernel(mha_masks) -> np.ndarray:
    global LAST_RESULT
    from concourse.bass_utils import run_bass_kernel_spmd

    xfull = np.ascontiguousarray(np.asarray(mha_masks, dtype=np.float32))
    assert xfull.shape == (B, H, N, N), xfull.shape

    nc = build_program()
    ident = np.eye(P, dtype=np.float32)
    in_maps = [
        {"x": xfull[i * BPC : (i + 1) * BPC], "ident": ident}
        for i in range(NCORES)
    ]
    import os

    kw = {}
    if os.environ.get("KERNEL_TRACE_DIR"):
        kw = dict(trace=True, tmpdir=os.environ["KERNEL_TRACE_DIR"])
    res = run_bass_kernel_spmd(nc, in_maps, core_ids=list(range(NCORES)), **kw)
    LAST_RESULT = res
    out = np.concatenate(
        [np.asarray(r["y"], dtype=np.float32) for r in res.results], axis=0
    )
    return out
